# revision 1
# baseline (speedup 1.0000x reference)
"""Trainium2 kernel for nn_CachedReadoutModel (PCA -> MLP -> species shift -> segment sum).

Strategy (8 NeuronCores, data-parallel over atoms):
  host:  fold PCA into layer 1 (W_eff = (W1 @ pca_components).T, b_eff = b1 - W_eff.T mean);
         slice 1M atoms into 8 ranges; within each range STABLE-SORT atoms by
         batch_map so each 128-atom tile spans <= 32 consecutive graphs; stage x
         transposed (feature-major) in bf16; precompute per-tile segment matrices
         S[t] in [128 atoms, 32 local graphs] (0/1, fp16) from the sorted batch_map.
  core:  h = silu(W_eff.T x + b_eff); mlp = h . w2            (bf16/fp16 matmuls)
         tot = mlp + (shifts + b2)[argmax(node_attrs)]         (exact argmax on DVE)
         split tot = tot_hi + tot_lo (fp16-exact pieces)
         per tile: out[0:32, 2t:2t+2] = S[t]^T @ [tot_hi | tot_lo]   (PE, N=2)
  host:  scatter-add the per-tile per-local-graph partial sums into delta[16384]
         (<= 18k values per core), sum cores, final = base_energy + delta.
"""

import os
import sys

for _p in ("/opt/trn_rl_repo", "/root/.axon_site/_ro/trn_rl_repo"):
    if os.path.isdir(_p) and _p not in sys.path:
        sys.path.insert(0, _p)

from contextlib import ExitStack

import numpy as np
import ml_dtypes

import concourse.bass as bass
import concourse.tile as tile
from concourse import bacc, mybir
from concourse._compat import with_exitstack
from concourse.bass_utils import run_bass_kernel_spmd

dt = mybir.dt
Alu = mybir.AluOpType
Act = mybir.ActivationFunctionType

N_ATOMS = 1_000_000
N_GRAPHS = 16384
NS = 10
N_CORES = 8
T = 992  # tiles of 128 atoms per core; A = 126976 >= ceil(1e6/8)
A = 128 * T
GPT = 32  # default max graphs per 128-atom tile (sorted); host adapts via _pick_gpt
TRUNC_MASK = int(~np.int32(0x1FFF))  # keep 10 explicit mantissa bits -> fp16-exact

_PROGRAM_CACHE = {}


@with_exitstack
def _emit_body(ctx: ExitStack, tc, T, ins, e_out, gpt=GPT, cpath_chunks=16, silu_mode="act"):
    nc = tc.nc
    assert T % 16 == 0
    n_super = T // 16

    const = ctx.enter_context(tc.tile_pool(name="const", bufs=1))
    cpath = ctx.enter_context(tc.tile_pool(name="cpath", bufs=2))
    xpool = ctx.enter_context(tc.tile_pool(name="xpool", bufs=3))
    spool = ctx.enter_context(tc.tile_pool(name="spool", bufs=3))
    work = ctx.enter_context(tc.tile_pool(name="work", bufs=3))
    hps = ctx.enter_context(tc.tile_pool(name="hps", bufs=2, space="PSUM"))
    eps = ctx.enter_context(tc.tile_pool(name="eps", bufs=1, space="PSUM"))

    def load_const(name, shape, dtype):
        t = const.tile(shape, dtype, tag=name)
        nc.sync.dma_start(t[:], ins[name])
        return t

    wa = load_const("wa", [128, 128], dt.bfloat16)
    wb = load_const("wb", [64, 128], dt.bfloat16)
    w2c = load_const("w2c", [128, 1], dt.float16)
    beff = load_const("beff", [128, 1], dt.float32)
    shiftsb = load_const("shiftsb", [128, NS], dt.float32)
    wpow = load_const("wpow", [128, NS], dt.float32)
    iota10n = load_const("iota10n", [128, NS], dt.float32)

    # --- c table: c[p, t] = (shifts + b2)[argmax_s na[p, t, :]] (exact first-index) ---
    # emitted in chunks interleaved with the main loop so DVE work overlaps PE work
    c_all = const.tile([128, T], dt.float32)
    assert T % cpath_chunks == 0
    Tc = T // cpath_chunks

    def emit_cpath_chunk(ci):
        nat_c = cpath.tile([128, Tc * NS], dt.float32, tag="natc")
        nc.sync.dma_start(nat_c[:], ins["nat"][:, ci * Tc * NS : (ci + 1) * Tc * NS])
        nat3 = nat_c[:].rearrange("p (t s) -> p t s", s=NS)
        mx = cpath.tile([128, Tc], dt.float32, tag="mx")
        nc.vector.tensor_reduce(out=mx[:], in_=nat3, op=Alu.max, axis=mybir.AxisListType.X)
        eq = cpath.tile([128, Tc * NS], dt.float32, tag="eq")
        eq3 = eq[:].rearrange("p (t s) -> p t s", s=NS)
        nc.vector.tensor_tensor(eq3, nat3, mx[:].unsqueeze(-1).broadcast_to([128, Tc, NS]), Alu.is_equal)
        rw = cpath.tile([128, Tc * NS], dt.float32, tag="rw")
        rw3 = rw[:].rearrange("p (t s) -> p t s", s=NS)
        nc.vector.tensor_tensor(rw3, eq3, wpow[:].unsqueeze(1).broadcast_to([128, Tc, NS]), Alu.mult)
        r = cpath.tile([128, Tc], dt.float32, tag="r")
        nc.vector.tensor_reduce(out=r[:], in_=rw3, op=Alu.add, axis=mybir.AxisListType.X)
        em_i = cpath.tile([128, Tc], dt.int32, tag="emi")
        nc.vector.tensor_scalar(em_i[:], r[:].bitcast(dt.int32), 23, None, Alu.logical_shift_right)
        em = cpath.tile([128, Tc], dt.float32, tag="em")
        nc.vector.tensor_scalar(em[:], em_i[:], 136, None, Alu.subtract)
        eq2 = cpath.tile([128, Tc * NS], dt.float32, tag="eq2")
        eq23 = eq2[:].rearrange("p (t s) -> p t s", s=NS)
        nc.vector.tensor_tensor(
            eq23,
            iota10n[:].unsqueeze(1).broadcast_to([128, Tc, NS]),
            em[:].unsqueeze(-1).broadcast_to([128, Tc, NS]),
            Alu.is_equal,
        )
        cw = cpath.tile([128, Tc * NS], dt.float32, tag="cw")
        cw3 = cw[:].rearrange("p (t s) -> p t s", s=NS)
        nc.vector.tensor_tensor(cw3, eq23, shiftsb[:].unsqueeze(1).broadcast_to([128, Tc, NS]), Alu.mult)
        nc.vector.tensor_reduce(out=c_all[:, ci * Tc : (ci + 1) * Tc], in_=cw3, op=Alu.add, axis=mybir.AxisListType.X)

    # --- main loop over superblocks of 2048 atoms (16 tiles) ---
    # one 4-bank PSUM tile: cols [0, 2T) = per-tile segment sums, cols [2T, 2T+32) = two mlp slots
    assert 2 * T + 32 <= 2048
    psum_all = eps.tile([128, 2048], dt.float32)
    e_ps = psum_all[:, 0 : 2 * T]
    next_chunk = 0
    for s in range(n_super):
        while next_chunk < cpath_chunks and s >= (next_chunk * n_super) // cpath_chunks - 2:
            emit_cpath_chunk(next_chunk)
            next_chunk += 1
        a0 = s * 2048
        x1 = xpool.tile([128, 2048], dt.bfloat16, tag="x1")
        nc.sync.dma_start(x1[:], ins["xt1"][:, a0 : a0 + 2048])
        x2 = xpool.tile([64, 2048], dt.bfloat16, tag="x2")
        nc.sync.dma_start(x2[:], ins["xt2"][:, a0 : a0 + 2048])
        if gpt <= 64 and s == 0:
            # HAM warm-up: ~10us of dense array work nudges the PE clock gate
            # toward 2.4 GHz; scratch output lands in psum rows 64..127 of the
            # segment area, which the host never reads.
            for w in range(24):
                nc.tensor.matmul(psum_all[64:128, 0:512], wa[:, 0:64], x1[:, 0:512], start=True, stop=True)
        st = spool.tile([128, 16 * gpt], dt.float16, tag="st")
        nc.sync.dma_start(st[:], ins["seg"][:, s * 16 * gpt : (s + 1) * 16 * gpt])
        mlp_ps = psum_all[:, 2 * T + 16 * (s % 2) : 2 * T + 16 * (s % 2) + 16]
        for half in range(2):
            h_ps = hps.tile([128, 1024], dt.float32)
            for q in range(2):
                sl = slice((2 * half + q) * 512, (2 * half + q + 1) * 512)
                out = h_ps[:, q * 512 : (q + 1) * 512]
                nc.tensor.matmul(out, wa[:], x1[:, sl], start=True, stop=False)
                nc.tensor.matmul(out, wb[:], x2[:, sl], start=False, stop=True)
            silu = work.tile([128, 1024], dt.float16, tag="silu")
            if silu_mode == "act":
                nc.scalar.activation(silu[:], h_ps[:], Act.Silu, bias=beff[:], scale=1.0)
            else:
                sg = work.tile([128, 1024], dt.float32, tag="sg")
                nc.scalar.activation(sg[:], h_ps[:], Act.Sigmoid, bias=beff[:], scale=1.0)
                hb = work.tile([128, 1024], dt.float32, tag="hb")
                nc.scalar.activation(hb[:], h_ps[:], Act.Identity, bias=beff[:], scale=1.0)
                nc.vector.tensor_tensor(silu[:], hb[:], sg[:], Alu.mult)
            for j in range(8):
                nc.tensor.matmul(
                    mlp_ps[:, half * 8 + j : half * 8 + j + 1],
                    silu[:, j * 128 : (j + 1) * 128],
                    w2c[:],
                    start=True,
                    stop=True,
                )
        # tot = mlp + c; split into fp16-exact hi + residual; interleave [hi|lo] pairs
        tot = work.tile([128, 16], dt.float32, tag="tot")
        nc.vector.tensor_tensor(tot[:], mlp_ps[:], c_all[:, s * 16 : (s + 1) * 16], Alu.add)
        tothi = work.tile([128, 16], dt.int32, tag="tothi")
        nc.vector.tensor_scalar(tothi[:], tot[:].bitcast(dt.int32), TRUNC_MASK, None, Alu.bitwise_and)
        totmov = work.tile([128, 32], dt.float16, tag="totmov")
        tm = totmov[:].rearrange("p (t two) -> p t two", two=2)
        nc.vector.tensor_copy(tm[:, :, 0], tothi[:].bitcast(dt.float32))
        nc.vector.tensor_tensor(tm[:, :, 1], tot[:], tothi[:].bitcast(dt.float32), Alu.subtract)
        for k in range(16):
            t = s * 16 + k
            nc.tensor.matmul(
                e_ps[0:gpt, 2 * t : 2 * t + 2],
                st[:, k * gpt : (k + 1) * gpt],
                totmov[:, 2 * k : 2 * k + 2],
                start=True,
                stop=True,
            )

    e_sb = const.tile([gpt, 2 * T], dt.float32)
    nc.vector.tensor_copy(e_sb[:], e_ps[0:gpt, :])
    nc.sync.dma_start(e_out, e_sb[:])


def _build_program(T, gpt=GPT, cpath_chunks=16, silu_mode="act"):
    A_ = 128 * T
    nc = bacc.Bacc("TRN2", target_bir_lowering=False, debug=False)
    shapes = {
        "xt1": ([128, A_], dt.bfloat16),
        "xt2": ([64, A_], dt.bfloat16),
        "seg": ([128, T * gpt], dt.float16),
        "nat": ([128, T * NS], dt.float32),
        "wa": ([128, 128], dt.bfloat16),
        "wb": ([64, 128], dt.bfloat16),
        "w2c": ([128, 1], dt.float16),
        "beff": ([128, 1], dt.float32),
        "shiftsb": ([128, NS], dt.float32),
        "wpow": ([128, NS], dt.float32),
        "iota10n": ([128, NS], dt.float32),
    }
    ins = {name: nc.declare_dram_parameter(name, list(sh), d, isOutput=False).ap() for name, (sh, d) in shapes.items()}
    e_out = nc.declare_dram_parameter("e_out", [gpt, 2 * T], dt.float32, isOutput=True).ap()
    with tile.TileContext(nc) as tc:
        _emit_body(tc, T, ins, e_out, gpt=gpt, cpath_chunks=cpath_chunks, silu_mode=silu_mode)
    nc.finalize()
    return nc


def _stage_params(pca_mean, pca_components, W1, b1, W2, b2, shifts):
    W_eff = (W1.astype(np.float64) @ pca_components.astype(np.float64)).T  # [192, 128]
    b_eff = b1.astype(np.float64) - W_eff.T @ pca_mean.astype(np.float64)
    W_eff = W_eff.astype(np.float32)
    bf = ml_dtypes.bfloat16
    return {
        "wa": np.ascontiguousarray(W_eff[:128]).astype(bf),
        "wb": np.ascontiguousarray(W_eff[128:]).astype(bf),
        "w2c": np.ascontiguousarray(W2.reshape(128, 1)).astype(np.float16),
        "beff": b_eff.astype(np.float32).reshape(128, 1),
        "shiftsb": np.broadcast_to((shifts + b2[0]).astype(np.float32), (128, NS)).copy(),
        "wpow": np.broadcast_to((2.0 ** (9 - np.arange(NS))).astype(np.float32), (128, NS)).copy(),
        "iota10n": np.broadcast_to((-np.arange(NS)).astype(np.float32), (128, NS)).copy(),
    }


def _stage_core_inputs(x_c, na_c, bm_c, gpt=GPT):
    """Sort one core's atoms by graph, pad to A, build device arrays + merge map."""
    n = x_c.shape[0]
    bf = ml_dtypes.bfloat16
    perm = np.argsort(bm_c, kind="stable")
    bm_s = bm_c[perm]

    xt = np.zeros((192, A), dtype=bf)
    xt[:, :n] = x_c[perm].T.astype(bf)
    nat = np.zeros((A, NS), dtype=np.float32)
    nat[:n] = na_c[perm]
    nat = np.ascontiguousarray(nat.reshape(T, 128, NS).transpose(1, 0, 2).reshape(128, T * NS))

    # segment matrices: new-graph flags / local ranks within each tile
    a_idx = np.arange(n)
    f = np.empty(n, dtype=bool)
    f[0] = True
    f[1:] = bm_s[1:] != bm_s[:-1]
    f |= a_idx % 128 == 0
    tile_of = a_idx // 128
    seg_start_rank = np.cumsum(f) - 1
    first_in_tile = np.searchsorted(tile_of, np.arange(T), side="left")
    # rank within tile = cumulative new-graph count since tile start
    base = seg_start_rank[np.minimum(first_in_tile, n - 1)]
    rank = seg_start_rank - base[tile_of]
    if n:
        assert rank.max() < gpt, f"graphs per tile exceeded {gpt}: {rank.max() + 1}"
    seg = np.zeros((T, 128, gpt), dtype=np.float16)
    seg[tile_of, a_idx % 128, rank] = 1.0
    seg = np.ascontiguousarray(seg.transpose(1, 0, 2).reshape(128, T * gpt))

    merge_tile = tile_of[f[:n]]
    merge_rank = rank[f[:n]]
    merge_graph = bm_s[f[:n]]
    return (
        {
            "xt1": np.ascontiguousarray(xt[:128]),
            "xt2": np.ascontiguousarray(xt[128:]),
            "seg": seg,
            "nat": nat,
        },
        (merge_tile.astype(np.int64), merge_rank.astype(np.int64), merge_graph.astype(np.int64)),
    )


def _get_program(gpt):
    key = (T, gpt, "act")
    if key not in _PROGRAM_CACHE:
        _PROGRAM_CACHE[key] = _build_program(T, gpt=gpt, silu_mode="act")
    return _PROGRAM_CACHE[key]


def _max_graphs_per_tile(bm_c):
    bm_s = np.sort(bm_c)
    n = len(bm_s)
    if n == 0:
        return 1
    f = np.empty(n, dtype=bool)
    f[0] = True
    f[1:] = bm_s[1:] != bm_s[:-1]
    f |= np.arange(n) % 128 == 0
    ranks = np.cumsum(f) - 1
    starts = ranks[np.arange(0, n, 128)]
    counts = np.diff(np.append(starts, ranks[-1] + 1))
    return int(counts.max())


def kernel(x, node_attrs, batch_map, base_energy, pca_mean, pca_components, W1, b1, W2, b2, shifts, _trace=False):
    x = np.asarray(x, dtype=np.float32)
    node_attrs = np.asarray(node_attrs, dtype=np.float32)
    batch_map = np.asarray(batch_map).astype(np.int64)
    base_energy = np.asarray(base_energy, dtype=np.float32)
    params = _stage_params(
        np.asarray(pca_mean, np.float32),
        np.asarray(pca_components, np.float32),
        np.asarray(W1, np.float32),
        np.asarray(b1, np.float32),
        np.asarray(W2, np.float32),
        np.asarray(b2, np.float32),
        np.asarray(shifts, np.float32),
    )

    n = x.shape[0]
    bounds = [min((n + N_CORES - 1) // N_CORES * c, n) for c in range(N_CORES + 1)]
    need = max(_max_graphs_per_tile(batch_map[bounds[c] : bounds[c + 1]]) for c in range(N_CORES))
    gpt = next(g for g in (32, 64, 128) if g >= need)
    in_maps, merges = [], []
    for c in range(N_CORES):
        s, e = bounds[c], bounds[c + 1]
        m, mg = _stage_core_inputs(x[s:e], node_attrs[s:e], batch_map[s:e], gpt=gpt)
        m.update(params)
        in_maps.append(m)
        merges.append(mg)

    nc = _get_program(gpt)
    res = run_bass_kernel_spmd(nc, in_maps, list(range(N_CORES)), trace=_trace)
    delta = np.zeros(N_GRAPHS, dtype=np.float64)
    for c in range(N_CORES):
        e_dev = np.asarray(res.results[c]["e_out"], dtype=np.float64)  # [gpt, 2T]
        mt, mr, mg = merges[c]
        vals = e_dev[mr, 2 * mt] + e_dev[mr, 2 * mt + 1]
        np.add.at(delta, mg, vals)
    delta = delta.astype(np.float32)
    final = base_energy + delta
    if _trace:
        kernel._last_result = res
    return final, delta



# revision 6
# speedup vs baseline: 1.0080x; 1.0080x over previous
"""Trainium2 kernel for nn_CachedReadoutModel (PCA -> MLP -> species shift -> segment sum).

v2 strategy (8 NeuronCores, data-parallel over atoms):
  host:  fold PCA into layer 1 (W_eff = (W1 @ pca_components).T, b_eff = b1 - W_eff.T mean);
         slice 1M atoms into 8 ranges; STABLE-SORT each range by batch_map; stage x
         feature-major in fp8e4 (scaled x32 via W instead), seg matrices fp16,
         node_attrs fp32.
  core:  per 2048-atom superblock:
           h = Weff^T x via ONE fp8 DoubleRow matmul pass (K=192 = 96 partition pairs)
           silu = ACT(Silu, scale=1/32, bias=beff) -> fp8 SBUF
           mlp[atom] = silu_chunk^T @ w2c (16 small MMs; the LDW is the atom transposer)
           tot = mlp/64 + c (DVE), fp16
           seg: 4-way col-tiled matmuls S_t^T @ tot -> e_ps[32*(t%4):+32, t//4]
         cpath (exact argmax shift table) on DVE, interleaved.
         Software-pipelined: big MM of superblock s+1 is emitted before the small
         MMs of s, so the PE never stalls on ACT and HAM stays at 2.4 GHz.
  host:  scatter-add per-(tile,rank) partials into delta[16384], sum cores,
         final = base_energy + delta.
"""

import os
import sys

for _p in ("/opt/trn_rl_repo", "/root/.axon_site/_ro/trn_rl_repo"):
    if os.path.isdir(_p) and _p not in sys.path:
        sys.path.insert(0, _p)

from contextlib import ExitStack

import numpy as np
import ml_dtypes

import concourse.bass as bass
import concourse.tile as tile
from concourse import bacc, mybir
from concourse._compat import with_exitstack
from concourse.bass_utils import run_bass_kernel_spmd

dt = mybir.dt
Alu = mybir.AluOpType
Act = mybir.ActivationFunctionType
PerfMode = mybir.MatmulPerfMode

N_ATOMS = 1_000_000
N_GRAPHS = 16384
NS = 10
N_CORES = 8
T = 992  # tiles of 128 atoms per core; A = 126976 >= ceil(1e6/8)
A = 128 * T
GPT = 32
W_SCALE = 32.0  # W_eff staged x32 (fp8 range); undone by ACT scale=1/32
W2_SCALE = 1.0  # w2 kept bf16, no scaling needed

_PROGRAM_CACHE = {}


@with_exitstack
def _emit_body(ctx: ExitStack, tc, T, ins, e_out, gpt=GPT, cpath_chunks=8):
    nc = tc.nc
    assert T % 16 == 0
    n_super = T // 16
    grp = max(1, 128 // gpt)  # tiles packed per psum column via col-tiling
    ecols = (T + grp - 1) // grp

    const = ctx.enter_context(tc.tile_pool(name="const", bufs=1))
    cpath = ctx.enter_context(tc.tile_pool(name="cpath", bufs=2))
    xpool = ctx.enter_context(tc.tile_pool(name="xpool", bufs=5))
    spool = ctx.enter_context(tc.tile_pool(name="spool", bufs=4))
    work = ctx.enter_context(tc.tile_pool(name="work", bufs=3))
    hps = ctx.enter_context(tc.tile_pool(name="hps", bufs=3, space="PSUM"))
    eps = ctx.enter_context(tc.tile_pool(name="eps", bufs=1, space="PSUM"))

    def load_const(name, shape, dtype):
        t = const.tile(shape, dtype, tag=name)
        nc.sync.dma_start(t[:], ins[name])
        return t

    wdr = load_const("wdr", [96, 256], dt.float8e4)  # [96, (2,128)] feature pairs
    w2c = load_const("w2c", [128, 1], dt.bfloat16)
    beff = load_const("beff", [128, 1], dt.float32)
    shiftsb = load_const("shiftsb", [128, NS], dt.float32)
    wpow = load_const("wpow", [128, NS], dt.float32)
    iota10n = load_const("iota10n", [128, NS], dt.float32)

    # --- c table: c[p, t] = (shifts + b2)[argmax_s na[p, t, :]] (exact first-index) ---
    c_all = const.tile([128, T], dt.float32)
    assert T % cpath_chunks == 0
    Tc = T // cpath_chunks

    def cpath_chunk_steps(ci):
        nat_c = cpath.tile([128, Tc * NS], dt.float32, tag="natc")
        nc.sync.dma_start(nat_c[:], ins["nat"][:, ci * Tc * NS : (ci + 1) * Tc * NS])
        nat3 = nat_c[:].rearrange("p (t s) -> p t s", s=NS)
        mx = cpath.tile([128, Tc], dt.float32, tag="mx")
        nc.vector.tensor_reduce(out=mx[:], in_=nat3, op=Alu.max, axis=mybir.AxisListType.X)
        yield
        eq = cpath.tile([128, Tc * NS], dt.float32, tag="eq")
        eq3 = eq[:].rearrange("p (t s) -> p t s", s=NS)
        nc.vector.tensor_tensor(eq3, nat3, mx[:].unsqueeze(-1).broadcast_to([128, Tc, NS]), Alu.is_equal)
        yield
        rw = cpath.tile([128, Tc * NS], dt.float32, tag="rw")
        rw3 = rw[:].rearrange("p (t s) -> p t s", s=NS)
        nc.vector.tensor_tensor(rw3, eq3, wpow[:].unsqueeze(1).broadcast_to([128, Tc, NS]), Alu.mult)
        yield
        r = cpath.tile([128, Tc], dt.float32, tag="r")
        nc.vector.tensor_reduce(out=r[:], in_=rw3, op=Alu.add, axis=mybir.AxisListType.X)
        em_i = cpath.tile([128, Tc], dt.int32, tag="emi")
        nc.vector.tensor_scalar(em_i[:], r[:].bitcast(dt.int32), 23, None, Alu.logical_shift_right)
        em = cpath.tile([128, Tc], dt.float32, tag="em")
        nc.vector.tensor_scalar(em[:], em_i[:], 136, None, Alu.subtract)
        yield
        eq2 = cpath.tile([128, Tc * NS], dt.float32, tag="eq2")
        eq23 = eq2[:].rearrange("p (t s) -> p t s", s=NS)
        nc.vector.tensor_tensor(
            eq23,
            iota10n[:].unsqueeze(1).broadcast_to([128, Tc, NS]),
            em[:].unsqueeze(-1).broadcast_to([128, Tc, NS]),
            Alu.is_equal,
        )
        yield
        cw = cpath.tile([128, Tc * NS], dt.float32, tag="cw")
        cw3 = cw[:].rearrange("p (t s) -> p t s", s=NS)
        nc.vector.tensor_tensor(cw3, eq23, shiftsb[:].unsqueeze(1).broadcast_to([128, Tc, NS]), Alu.mult)
        yield
        nc.vector.tensor_reduce(out=c_all[:, ci * Tc : (ci + 1) * Tc], in_=cw3, op=Alu.add, axis=mybir.AxisListType.X)
        yield

    def cpath_all_steps():
        for ci in range(cpath_chunks):
            yield from cpath_chunk_steps(ci)

    # --- PSUM: e/mlp tile (1-2 banks) + h pool (3 x 2 banks) ---
    emlp = eps.tile([128, ((ecols + 32 + 127) // 128) * 128], dt.float32)
    e_ps = emlp[:, 0:ecols]
    mlp_base = ecols

    # per-superblock state for the software pipeline
    silu_sb = [None] * n_super
    st_sb = [None] * n_super
    x_sb = [None] * n_super

    def emit_dma(s):
        # x: [96, (2,2048)] fp8 -- two feature planes side by side in one tile
        xt = xpool.tile([96, 4096], dt.float8e4, tag="x")
        nc.sync.dma_start(xt[:], ins["xeo"][:, s * 4096 : (s + 1) * 4096])
        x_sb[s] = xt
        if s % 2 == 0:  # st for two superblocks per DMA
            st = spool.tile([128, 32 * gpt], dt.float16, tag="st")
            nc.sync.dma_start(st[:], ins["seg"][:, s * 16 * gpt : (s + 2) * 16 * gpt])
            st_sb[s] = st
            st_sb[s + 1] = st

    def emit_big_half(s, half, warmup=False):
        # h[128 hidden, 1024 atoms] via DoubleRow fp8: one K=192 pass
        h_ps = hps.tile([128, 1024], dt.float32)
        xt = x_sb[s]
        lhsT = wdr[:].rearrange("p (two m) -> p two m", two=2)
        x3 = xt[:].rearrange("p (two n) -> p two n", two=2)
        if warmup:
            # dense PE burst on a scratch psum slice to push HAM toward
            # 2.4 GHz before the steady-state pipeline begins (~4us)
            wsc = emlp[:, mlp_base + 32 :]
            for w in range(16):
                nc.tensor.matmul(
                    wsc[:, 0:48],
                    lhsT,
                    x3[:, :, 0:48],
                    start=True,
                    stop=True,
                    perf_mode=PerfMode.DoubleRow,
                )
        for q in range(2):
            out = h_ps[:, q * 512 : (q + 1) * 512]
            rhs = x3[:, :, half * 1024 + q * 512 : half * 1024 + (q + 1) * 512]
            nc.tensor.matmul(out, lhsT, rhs, start=True, stop=True, perf_mode=PerfMode.DoubleRow)
        return h_ps

    def emit_act_half(s, half, h_ps):
        if half == 0:
            silu = work.tile([128, 2048], dt.bfloat16, tag="silu")
            silu_sb[s] = silu
        nc.scalar.activation(
            silu_sb[s][:, half * 1024 : (half + 1) * 1024],
            h_ps[:],
            Act.Silu,
            bias=beff[:],
            scale=1.0 / W_SCALE,
        )

    tot_sb = [None] * n_super

    def emit_w2c(s):
        silu = silu_sb[s]
        mlp_ps = emlp[:, mlp_base + 16 * (s % 2) : mlp_base + 16 * (s % 2) + 16]
        for j in range(16):
            nc.tensor.matmul(
                mlp_ps[:, j : j + 1],
                silu[:, j * 128 : (j + 1) * 128],
                w2c[:],
                start=True,
                stop=True,
            )
        totmov = work.tile([128, 16], dt.float16, tag="totmov")
        nc.vector.tensor_tensor(totmov[:], mlp_ps[:], c_all[:, s * 16 : (s + 1) * 16], Alu.add)
        tot_sb[s] = totmov
        silu_sb[s] = None
        x_sb[s] = None

    def emit_seg(s):
        totmov = tot_sb[s]
        st = st_sb[s]
        soff = 0 if s % 2 == 0 else 16 * gpt
        for k in range(16):
            t = s * 16 + k
            i = t % grp
            nc.tensor.matmul(
                e_ps[i * gpt : (i + 1) * gpt, t // grp : t // grp + 1],
                st[:, soff + k * gpt : soff + (k + 1) * gpt],
                totmov[:, k : k + 1],
                start=True,
                stop=True,
                tile_position=(0, i * gpt) if grp > 1 else None,
            )
        tot_sb[s] = None
        st_sb[s] = None

    # --- software-pipelined main loop: big(s) | w2c(s-1) | seg(s-2) ---
    PREF = 4  # DMA prefetch depth (superblocks)
    cp = cpath_all_steps()

    def pull_cpath(k):
        for _ in range(k):
            next(cp, None)

    for s in range(min(PREF, n_super)):
        emit_dma(s)
    pull_cpath(7)  # chunk 0 ahead of the first totmov
    for s in range(n_super):
        if s + PREF < n_super:
            emit_dma(s + PREF)
        for half in range(2):
            h_ps = emit_big_half(s, half, warmup=(s == 0 and half == 0))
            emit_act_half(s, half, h_ps)
        if s >= 1:
            emit_w2c(s - 1)
        if s >= 2:
            emit_seg(s - 2)
        pull_cpath(2 if s % 2 == 0 else 1)
    emit_w2c(n_super - 1)
    emit_seg(n_super - 2)
    emit_seg(n_super - 1)
    pull_cpath(10 * cpath_chunks)

    e_sb = const.tile([128, ecols], dt.float32)
    nc.vector.tensor_copy(e_sb[:], e_ps)
    nc.sync.dma_start(e_out, e_sb[:])


def _build_program(T, gpt=GPT, cpath_chunks=8):
    nc = bacc.Bacc("TRN2", target_bir_lowering=False, debug=False)
    A_ = 128 * T
    grp = max(1, 128 // gpt)
    ecols = (T + grp - 1) // grp
    shapes = {
        "xeo": ([96, 2 * A_], dt.float8e4),
        "seg": ([128, T * gpt], dt.float16),
        "nat": ([128, T * NS], dt.float32),
        "wdr": ([96, 256], dt.float8e4),
        "w2c": ([128, 1], dt.bfloat16),
        "beff": ([128, 1], dt.float32),
        "shiftsb": ([128, NS], dt.float32),
        "wpow": ([128, NS], dt.float32),
        "iota10n": ([128, NS], dt.float32),
    }
    ins = {name: nc.declare_dram_parameter(name, list(sh), d, isOutput=False).ap() for name, (sh, d) in shapes.items()}
    e_out = nc.declare_dram_parameter("e_out", [128, ecols], dt.float32, isOutput=True).ap()
    with tile.TileContext(nc) as tc:
        _emit_body(tc, T, ins, e_out, gpt=gpt, cpath_chunks=cpath_chunks)
    nc.finalize()
    return nc


def _stage_params(pca_mean, pca_components, W1, b1, W2, b2, shifts):
    W_eff = (W1.astype(np.float64) @ pca_components.astype(np.float64)).T  # [192, 128]
    b_eff = b1.astype(np.float64) - W_eff.T @ pca_mean.astype(np.float64)
    f8 = ml_dtypes.float8_e4m3fn
    # wdr[ki, j, m] = W_eff[2ki+j, m] * W_SCALE  -> [96, 256]
    wdr = (W_eff * W_SCALE).astype(np.float32).reshape(96, 2, 128).reshape(96, 256)
    return {
        "wdr": np.ascontiguousarray(wdr).astype(f8),
        "w2c": np.ascontiguousarray(W2.reshape(128, 1)).astype(ml_dtypes.bfloat16),
        "beff": b_eff.astype(np.float32).reshape(128, 1),
        "shiftsb": np.broadcast_to((shifts + b2[0]).astype(np.float32), (128, NS)).copy(),
        "wpow": np.broadcast_to((2.0 ** (9 - np.arange(NS))).astype(np.float32), (128, NS)).copy(),
        "iota10n": np.broadcast_to((-np.arange(NS)).astype(np.float32), (128, NS)).copy(),
    }


def _stage_core_inputs(x_c, na_c, bm_c, gpt=GPT):
    """Sort one core's atoms by graph, pad to A, build device arrays + merge map."""
    n = x_c.shape[0]
    f8 = ml_dtypes.float8_e4m3fn
    perm = np.argsort(bm_c, kind="stable")
    bm_s = bm_c[perm]

    xt = np.zeros((192, A), dtype=f8)
    xt[:, :n] = x_c[perm].T.astype(f8)
    # xeo: per-superblock interleave [even-plane block | odd-plane block], one DMA/superblock
    xeo = np.empty((96, 2 * A), dtype=f8)
    xev = xt[0::2].reshape(96, A // 2048, 2048)
    xov = xt[1::2].reshape(96, A // 2048, 2048)
    xv = xeo.reshape(96, A // 2048, 2, 2048)
    xv[:, :, 0, :] = xev
    xv[:, :, 1, :] = xov
    nat = np.zeros((A, NS), dtype=np.float32)
    nat[:n] = na_c[perm]
    nat = np.ascontiguousarray(nat.reshape(T, 128, NS).transpose(1, 0, 2).reshape(128, T * NS))

    a_idx = np.arange(n)
    f = np.empty(n, dtype=bool)
    f[0] = True
    f[1:] = bm_s[1:] != bm_s[:-1]
    f |= a_idx % 128 == 0
    tile_of = a_idx // 128
    seg_start_rank = np.cumsum(f) - 1
    first_in_tile = np.searchsorted(tile_of, np.arange(T), side="left")
    base = seg_start_rank[np.minimum(first_in_tile, n - 1)]
    rank = seg_start_rank - base[tile_of]
    if n:
        assert rank.max() < gpt, f"graphs per tile exceeded {gpt}: {rank.max() + 1}"
    seg = np.zeros((T, 128, gpt), dtype=np.float16)
    seg[tile_of, a_idx % 128, rank] = 1.0
    seg = np.ascontiguousarray(seg.transpose(1, 0, 2).reshape(128, T * gpt))

    merge_tile = tile_of[f[:n]]
    merge_rank = rank[f[:n]]
    merge_graph = bm_s[f[:n]]
    return (
        {"xeo": xeo, "seg": seg, "nat": nat},
        (merge_tile.astype(np.int64), merge_rank.astype(np.int64), merge_graph.astype(np.int64)),
    )


def _get_program(gpt):
    key = (T, gpt, "v3")
    if key not in _PROGRAM_CACHE:
        _PROGRAM_CACHE[key] = _build_program(T, gpt=gpt)
    return _PROGRAM_CACHE[key]


def _max_graphs_per_tile(bm_c):
    bm_s = np.sort(bm_c)
    n = len(bm_s)
    if n == 0:
        return 1
    f = np.empty(n, dtype=bool)
    f[0] = True
    f[1:] = bm_s[1:] != bm_s[:-1]
    f |= np.arange(n) % 128 == 0
    ranks = np.cumsum(f) - 1
    starts = ranks[np.arange(0, n, 128)]
    counts = np.diff(np.append(starts, ranks[-1] + 1))
    return int(counts.max())


def kernel(x, node_attrs, batch_map, base_energy, pca_mean, pca_components, W1, b1, W2, b2, shifts, _trace=False):
    x = np.asarray(x, dtype=np.float32)
    node_attrs = np.asarray(node_attrs, dtype=np.float32)
    batch_map = np.asarray(batch_map).astype(np.int64)
    base_energy = np.asarray(base_energy, dtype=np.float32)
    params = _stage_params(
        np.asarray(pca_mean, np.float32),
        np.asarray(pca_components, np.float32),
        np.asarray(W1, np.float32),
        np.asarray(b1, np.float32),
        np.asarray(W2, np.float32),
        np.asarray(b2, np.float32),
        np.asarray(shifts, np.float32),
    )

    n = x.shape[0]
    bounds = [min((n + N_CORES - 1) // N_CORES * c, n) for c in range(N_CORES + 1)]
    need = max(_max_graphs_per_tile(batch_map[bounds[c] : bounds[c + 1]]) for c in range(N_CORES))
    gpt = next(g for g in (32, 64, 128) if g >= need)
    in_maps, merges = [], []
    for c in range(N_CORES):
        s, e = bounds[c], bounds[c + 1]
        m, mg = _stage_core_inputs(x[s:e], node_attrs[s:e], batch_map[s:e], gpt=gpt)
        m.update(params)
        in_maps.append(m)
        merges.append(mg)

    nc = _get_program(gpt)
    res = run_bass_kernel_spmd(nc, in_maps, list(range(N_CORES)), trace=_trace)
    grp = max(1, 128 // gpt)
    delta = np.zeros(N_GRAPHS, dtype=np.float64)
    for c in range(N_CORES):
        e_dev = np.asarray(res.results[c]["e_out"], dtype=np.float64)  # [128, ecols]
        mt, mr, mg = merges[c]
        vals = e_dev[(mt % grp) * gpt + mr, mt // grp]
        np.add.at(delta, mg, vals)
    delta = delta.astype(np.float32)
    final = base_energy + delta
    if _trace:
        kernel._last_result = res
    return final, delta
